# revision 17
# baseline (speedup 1.0000x reference)
"""Trainium2 Bass kernel for nn_MlpwithSOMModule (pairwise-concat MLP + max/mask/sum).

Reference computation (B=8, C=4, T=128, D=64, H=128, G=B*C=32):
  entity  = input[:,:,1] -> [G,T,D];  context = input[:,:,0] -> [G,T,D]
  mask    = (context[:,:,0] != 0)                         [G,T]
  x[g,i,j] = concat(context[g,i], entity[g,j])            [G,T,T,2D]
  for l in 0..5: x = tanh(x @ Ws[l] + bs[l])
  score  = (x @ W_out + b_out)[...,0]                     [G,T,T]
  out[g] = sum_i( max_j(score[g,i,j]) * mask[g,i] )       [G]

Sharding: data-parallel over G across 8 cores (4 groups/core); weights
replicated.  On-chip layout is feature-major ([128 features, pairs]) so every
MLP layer is one stationary-weight matmul.  Layer 0 uses the concat split:
  x0 = ctx_i @ W0[:D] + ent_j @ W0[D:]  ->  A[:,i] + Bb[:,j]
with A, Bb computed once per group as [128,128] matrices.

Matmul operands and activations run in bf16 (PSUM accumulation and all
bias/score/max/sum arithmetic stay fp32); the mask is computed from an fp32
slice of the context so (x != 0) is exact.
"""

import numpy as np
import ml_dtypes

import concourse.bacc as bacc
import concourse.mybir as mybir
import concourse.tile as tile
from concourse.bass_utils import run_bass_kernel_spmd

B, C, T, D = 8, 4, 128, 64
H = 2 * D          # 128
G = B * C          # 32 groups
N_CORES = 8
G_LOC = G // N_CORES   # 4 groups per core
NJ_CHUNK = 16          # j's per chunk
CHUNK = NJ_CHUNK * T   # 2048 pairs per chunk
N_CHUNKS = T // NJ_CHUNK  # 8 chunks per group

F32 = mybir.dt.float32
BF16 = mybir.dt.bfloat16
AF = mybir.ActivationFunctionType
ALU = mybir.AluOpType
AX = mybir.AxisListType

_cached_nc = None


def _build_program():
    nc = bacc.Bacc("TRN2", target_bir_lowering=False, debug=False,
                   num_devices=N_CORES)

    ctxT_d = nc.dram_tensor("ctxT", [G_LOC, D, T], BF16, kind="ExternalInput")
    entT_d = nc.dram_tensor("entT", [G_LOC, D, T], BF16, kind="ExternalInput")
    ctx0_d = nc.dram_tensor("ctx0", [G_LOC, T, 1], F32, kind="ExternalInput")
    ws_d = nc.dram_tensor("Ws", [6, H, H], BF16, kind="ExternalInput")
    w0b_d = nc.dram_tensor("w0b", [D, H], BF16, kind="ExternalInput")
    bsT_d = nc.dram_tensor("bsT", [H, 6], F32, kind="ExternalInput")
    wout_d = nc.dram_tensor("wout", [H, 1], BF16, kind="ExternalInput")
    bout_d = nc.dram_tensor("bout", [T, 1], F32, kind="ExternalInput")
    out_d = nc.dram_tensor("out", [1, G_LOC], F32, kind="ExternalOutput")

    with tile.TileContext(nc) as tc:
        with (
            tc.tile_pool(name="consts", bufs=1) as consts,
            tc.tile_pool(name="zpool", bufs=4) as zpool,
            tc.tile_pool(name="hpool", bufs=6) as hpool,
            tc.tile_pool(name="small", bufs=4) as small,
            tc.tile_pool(name="psum", bufs=2, space="PSUM") as psum,
        ):
            ws_sb = consts.tile([H, 6 * H], BF16)
            for l in range(6):
                nc.sync.dma_start(ws_sb[:, l * H:(l + 1) * H], ws_d[l])
            w0b_sb = consts.tile([D, H], BF16)
            nc.sync.dma_start(w0b_sb[:], w0b_d[:])
            bsT_sb = consts.tile([H, 6], F32)
            nc.sync.dma_start(bsT_sb[:], bsT_d[:])
            wout_sb = consts.tile([H, 1], BF16)
            nc.sync.dma_start(wout_sb[:], wout_d[:])
            bout_sb = consts.tile([T, 1], F32)
            nc.sync.dma_start(bout_sb[:], bout_d[:])
            ones_sb = consts.tile([T, 1], F32)
            nc.vector.memset(ones_sb[:], 1.0)
            res_sb = consts.tile([1, G_LOC], F32)

            # Per-group setup, all upfront: A/Bb first-layer matrices, masks.
            a_sbs, bb_sbs, ctx0_sbs, rmax_sbs = [], [], [], []
            for g in range(G_LOC):
                ctxT_sb = consts.tile([D, T], BF16, tag=f"ctx{g}")
                entT_sb = consts.tile([D, T], BF16, tag=f"ent{g}")
                ctx0_sb = consts.tile([T, 1], F32, tag=f"ctx0_{g}")
                nc.sync.dma_start(ctxT_sb[:], ctxT_d[g])
                nc.sync.dma_start(entT_sb[:], entT_d[g])
                nc.sync.dma_start(ctx0_sb[:], ctx0_d[g])

                # A = (ctx @ W0_top).T : [H, T(i)];  Bb = (ent @ W0_bot).T + b0
                ps_ab = psum.tile([H, 4 * 512], F32, tag="mm")
                nc.tensor.matmul(ps_ab[:, 0:T], ws_sb[0:D, 0:H],
                                 ctxT_sb[:], start=True, stop=True)
                nc.tensor.matmul(ps_ab[:, 512:512 + T], w0b_sb[:],
                                 entT_sb[:], start=True, stop=True)
                a_sb = consts.tile([H, T], BF16, tag=f"a{g}")
                nc.vector.tensor_copy(a_sb[:], ps_ab[:, 0:T])
                bb_sb = consts.tile([H, T], F32, tag=f"bb{g}")
                nc.vector.tensor_scalar_add(bb_sb[:], ps_ab[:, 512:512 + T],
                                            bsT_sb[:, 0:1])
                rmax_sb = consts.tile([T, 1], F32, tag=f"rmax{g}")
                nc.vector.memset(rmax_sb[:], -1e30)
                a_sbs.append(a_sb)
                bb_sbs.append(bb_sb)
                ctx0_sbs.append(ctx0_sb)
                rmax_sbs.append(rmax_sb)

            def build_z(cc):
                """Layer 0 for global chunk cc: z[:, jl*T+i] = A[:,i]+Bb[:,j]."""
                g = cc // N_CHUNKS
                c = cc % N_CHUNKS
                z_sb = zpool.tile([H, CHUNK], BF16, tag="z")
                for jl in range(NJ_CHUNK):
                    j = c * NJ_CHUNK + jl
                    nc.vector.tensor_scalar_add(
                        z_sb[:, jl * T:(jl + 1) * T], a_sbs[g][:],
                        bb_sbs[g][:, j:j + 1])
                return z_sb

            TOT = G_LOC * N_CHUNKS  # 32 chunks, processed in pairs
            z_tiles = {0: build_z(0), 1: build_z(1)}

            for p in range(TOT // 2):
                cA, cB = 2 * p, 2 * p + 1
                h_cur = {}
                for cc in (cA, cB):
                    h_sb = hpool.tile([H, CHUNK], BF16, tag="h")
                    nc.scalar.activation(h_sb[:], z_tiles.pop(cc)[:], AF.Tanh)
                    h_cur[cc] = h_sb
                # prefetch next pair's layer-0 on DVE (ahead of the reduces)
                if cB + 2 < TOT:
                    z_tiles[cA + 2] = build_z(cA + 2)
                    z_tiles[cB + 2] = build_z(cB + 2)

                for l in range(1, 6):
                    for cc in (cA, cB):
                        ps = psum.tile([H, CHUNK], F32, tag="mm")
                        for q in range(4):
                            nc.tensor.matmul(
                                ps[:, q * 512:(q + 1) * 512],
                                ws_sb[:, l * H:(l + 1) * H],
                                h_cur[cc][:, q * 512:(q + 1) * 512],
                                start=True, stop=True)
                        h2_sb = hpool.tile([H, CHUNK], BF16, tag="h")
                        nc.scalar.activation(h2_sb[:], ps[:], AF.Tanh,
                                             bias=bsT_sb[:, l:l + 1])
                        h_cur[cc] = h2_sb

                for cc in (cA, cB):
                    g = cc // N_CHUNKS
                    # final layer, transposed: score col [T(i), 1] per j via
                    # stationary h-block x moving W_out
                    sc_ps = psum.tile([H, CHUNK], F32, tag="mm")
                    for jl in range(NJ_CHUNK):
                        nc.tensor.matmul(
                            sc_ps[:, jl:jl + 1],
                            h_cur[cc][:, jl * T:(jl + 1) * T],
                            wout_sb[:], start=True, stop=True)
                    # max over the 16 j's, fold into running max (both [T,1])
                    tmp_sb = small.tile([T, 1], F32, tag="tmp")
                    nc.vector.tensor_reduce(tmp_sb[:], sc_ps[:, 0:NJ_CHUNK],
                                            axis=AX.X, op=ALU.max)
                    nc.vector.tensor_max(rmax_sbs[g][:], rmax_sbs[g][:],
                                         tmp_sb[:])

            for g in range(G_LOC):
                # mask = (ctx[:,0] != 0); out = sum_i(mask*(rmax+b_out))
                mask_sb = small.tile([T, 1], F32, tag="mask")
                nc.vector.tensor_scalar(mask_sb[:], ctx0_sbs[g][:], 0.0, None,
                                        op0=ALU.not_equal)
                rb_sb = small.tile([T, 1], F32, tag="rb")
                nc.vector.tensor_scalar_add(rb_sb[:], rmax_sbs[g][:],
                                            bout_sb[:, 0:1])
                mm_sb = small.tile([T, 1], F32, tag="mmul")
                nc.vector.tensor_mul(mm_sb[:], rb_sb[:], mask_sb[:])
                # partition-axis sum via ones-matmul: [1,1] = mm.T @ ones
                sum_ps = psum.tile([H, CHUNK], F32, tag="mm")
                nc.tensor.matmul(sum_ps[0:1, 0:1], mm_sb[:], ones_sb[:],
                                 start=True, stop=True)
                nc.vector.tensor_copy(res_sb[0:1, g:g + 1], sum_ps[0:1, 0:1])

            nc.sync.dma_start(out_d[:], res_sb[:])

    nc.compile()
    return nc


def _get_nc():
    global _cached_nc
    if _cached_nc is None:
        _cached_nc = _build_program()
    return _cached_nc


def _bf16(a):
    return np.ascontiguousarray(a.astype(ml_dtypes.bfloat16))


def _prep_in_maps(input, Ws, bs, W_out, b_out):
    input = np.ascontiguousarray(np.asarray(input, dtype=np.float32))
    Ws = np.asarray(Ws, dtype=np.float32)
    bs = np.asarray(bs, dtype=np.float32)
    W_out = np.asarray(W_out, dtype=np.float32)
    b_out = np.asarray(b_out, dtype=np.float32)

    ctx = input[:, :, 0].reshape(G, T, D)
    ent = input[:, :, 1].reshape(G, T, D)
    ctxT = _bf16(ctx.transpose(0, 2, 1))                  # [G, D, T]
    entT = _bf16(ent.transpose(0, 2, 1))
    ctx0 = np.ascontiguousarray(ctx[:, :, 0]).reshape(G, T, 1)  # fp32
    ws_bf = _bf16(Ws)
    w0b = _bf16(Ws[0][D:H])
    bsT = np.ascontiguousarray(bs.T)                      # [H, 6]
    wout = _bf16(W_out)
    bout = np.broadcast_to(b_out.reshape(1, 1), (T, 1)).copy()

    in_maps = []
    for k in range(N_CORES):
        sl = slice(k * G_LOC, (k + 1) * G_LOC)
        in_maps.append({
            "ctxT": np.ascontiguousarray(ctxT[sl]),
            "entT": np.ascontiguousarray(entT[sl]),
            "ctx0": np.ascontiguousarray(ctx0[sl]),
            "Ws": ws_bf,
            "w0b": w0b,
            "bsT": bsT,
            "wout": wout,
            "bout": bout,
        })
    return in_maps


def run_traced(trace=False, **inputs):
    """Returns (output [G], exec_time_ns or None)."""
    nc = _get_nc()
    in_maps = _prep_in_maps(**inputs)
    res = run_bass_kernel_spmd(nc, in_maps, list(range(N_CORES)), trace=trace)
    out = np.concatenate([res.results[k]["out"].reshape(G_LOC)
                          for k in range(N_CORES)])
    return out, res.exec_time_ns


def kernel(**inputs) -> np.ndarray:
    out, _ = run_traced(trace=False, **inputs)
    return out
